# revision 21
# baseline (speedup 1.0000x reference)
"""GAT layer (4-head, 128-dim) on 8 Trainium2 NeuronCores.

v2 architecture (dma_gather-based; measured-primitive-driven):
  - Destination-range sharding: core c owns dst nodes [c*12544, (c+1)*12544).
    No output all-reduce; host concatenates disjoint shards.
  - Node table (replicated, built on device): 512B rows
    [wh fp16 x128 | s_src fp32 x4 (bitcast in fp16 slots) | zeros].
  - Edges are bucketed by src range (4 buckets of 25088 rows) so int16
    dma_gather indices reach the whole table; bucket q runs on SWDGE queue q.
  - Edge slots grouped into 128-edge blocks per (dst-tile, bucket) with baked
    per-(tile,bucket) block counts (max over cores -> one SPMD program).
  - Per gather instruction (<=8 blocks, <=1024 idx): gather X rows, compute
    s_dst per edge via one-hot-transpose matmul vs the tile's local s_dst
    window (no dst gather!), p = exp(leaky(ss+sd)), scale wh by per-head p,
    build one-hot of dst-within-tile, matmul-accumulate
    one_hot^T @ [wh*p | p] into the dst tile's PSUM accumulator.
  - Flush per dst tile: out = U / max(denom, eps), stream to HBM.
"""

import numpy as np
import sys

if "/opt/trn_rl_repo" not in sys.path:
    sys.path.insert(0, "/opt/trn_rl_repo")

# ---------------------------------------------------------------- constants
N_NODES = 100000
N_EDGES = 1600000
D = 128          # feature dim
H = 4            # heads
HD = 32          # head dim
NEG = 0.2        # leaky relu slope
R = 8            # cores
P = 128          # partitions
NBUK = 4         # src buckets (int16 reach)
ROW = 256        # table row, fp16 elems (512B)
TW = D + H       # X' row: [wh*p | p] = 132
MAXBLK = 8       # blocks per gather instruction (1024 idx ring limit)

_PROG_CACHE = {}


def _ceil_div(a, b):
    return (a + b - 1) // b


# ---------------------------------------------------------------- host prep
def host_prep(h, edge_index, W, a, npc, tg):
    """Index/layout prep only. Returns (per_core inputs, structure)."""
    n = h.shape[0]
    n_pad = R * npc
    nt = npc // P
    buk = n_pad // NBUK

    src = np.asarray(edge_index[0], dtype=np.int64)
    dst = np.asarray(edge_index[1], dtype=np.int64)
    e = src.shape[0]

    gtile = dst // P                        # 0 .. R*nt-1
    q = src // buk                          # bucket 0..3
    key = gtile * NBUK + q                  # (tile, bucket) group
    order = np.argsort(key, kind="stable")
    key_s = key[order]
    cnt = np.bincount(key_s, minlength=R * nt * NBUK).reshape(R, nt, NBUK)

    # baked per-(tile,bucket) block counts: max over cores, >=1
    bq = np.maximum(_ceil_div(cnt, P).max(axis=0), 1)      # [nt, NBUK]
    nb = int(bq.sum())                                     # blocks per core

    # block layout: supers of tg tiles; within super: bucket-major
    # block_meta[b] = (tile, bucket)
    block_meta = []
    blk_base = np.zeros((nt, NBUK), np.int64)
    for t0 in range(0, nt, tg):
        tt = min(tg, nt - t0)
        for qq in range(NBUK):
            for t in range(t0, t0 + tt):
                blk_base[t, qq] = len(block_meta)
                block_meta += [(t, qq)] * int(bq[t, qq])
    assert len(block_meta) == nb
    block_meta = np.array(block_meta, np.int64)            # [nb, 2]

    # gather instructions: runs of consecutive same-bucket blocks, <= MAXBLK
    instrs = []                                            # (q, b0, nblk)
    b = 0
    while b < nb:
        qq = block_meta[b, 1]
        b2 = b
        while (b2 < nb and block_meta[b2, 1] == qq and b2 - b < MAXBLK):
            b2 += 1
        instrs.append((int(qq), b, b2 - b))
        b = b2

    # slot assignment
    run_start = np.zeros(R * nt * NBUK + 1, np.int64)
    np.cumsum(cnt.reshape(-1), out=run_start[1:])
    rank = np.arange(e, dtype=np.int64) - run_start[key_s]
    core = key_s // (nt * NBUK)
    t_s = (key_s // NBUK) % nt
    q_s = key_s % NBUK
    block = blk_base[t_s, q_s] + rank // P
    lane = rank % P

    so = src[order]
    do = dst[order]

    idx16 = np.zeros((R, P, nb), np.int16)                 # bucket-relative
    colv = np.full((R, P, nb), -1.0, np.float16)
    idx16[core, lane, block] = (so - q_s * buk).astype(np.int16)
    colv[core, lane, block] = (do % P).astype(np.float16)

    # dma_gather idx wrapping: within each instruction, flat slot
    # i = (b - b0)*128 + lane -> wrapped [16, i//16], columns packed
    # consecutively across instructions; replicated to 128 partitions.
    idxw_cols = nb * P // 16
    idx_w = np.zeros((R, 16, idxw_cols), np.int16)
    # instruction k covers blocks [b0, b0+nblk): its flat index i maps to
    # column b0*8 + i//16 (each block contributes 8 columns).
    bflat = block * P + lane                               # global flat slot
    idx_w[core, bflat % 16, bflat // 16] = idx16[core, lane, block]
    idx_rep = np.repeat(idx_w, 8, axis=0).reshape(R, 128, idxw_cols)

    # col-by-free for the one-hot-transpose: colfree[:, b*128 + lane]
    # (replicated across partitions -- DVE can't broadcast partitions)
    colfree1 = np.full((R, nb * P), -1.0, np.float16)
    colfree1[core, bflat] = colv[core, lane, block]
    colfree = np.broadcast_to(colfree1[:, None, :],
                              (R, P, nb * P)).copy()

    # float layout transforms
    hT = np.zeros((D, n_pad), np.float32)
    hT[:, :n] = np.asarray(h, dtype=np.float32).T
    hT16 = hT.astype(np.float16)
    W16 = np.asarray(W, dtype=np.float16)
    WT32 = np.ascontiguousarray(np.asarray(W, dtype=np.float32).T)
    A32 = np.zeros((D, 2 * H), np.float32)
    aa = np.asarray(a, dtype=np.float32)
    for hh in range(H):
        A32[hh * HD:(hh + 1) * HD, hh] = aa[hh, :HD]
        A32[hh * HD:(hh + 1) * HD, H + hh] = aa[hh, HD:]
    iota2d = np.tile(np.arange(P, dtype=np.float16)[None, :], (P, 1))
    iota_col = np.arange(P, dtype=np.float16)[:, None]

    per_core = []
    for c in range(R):
        per_core.append({
            "hT16": hT16,
            "hT16s": np.ascontiguousarray(hT16[:, c * npc:(c + 1) * npc]),
            "W16": W16,
            "WT32": WT32,
            "A32": A32,
            "iota2d": iota2d,
            "iota_col": iota_col,
            "idxg": idx_rep[c],
            "colv": colv[c],
            "colfree": colfree[c],
        })
    struct = {"bq": bq, "nb": nb, "block_meta": block_meta,
              "instrs": instrs, "blk_base": blk_base}
    return per_core, struct


# ---------------------------------------------------------------- program
def build_program(npc, tg, struct, debug=False):
    from concourse import bass, bacc, tile
    import concourse.mybir as mybir

    f16 = mybir.dt.float16
    f32 = mybir.dt.float32
    i16 = mybir.dt.int16
    Alu = mybir.AluOpType
    Act = mybir.ActivationFunctionType

    n_pad = R * npc
    nt = npc // P
    nb = struct["nb"]
    block_meta = struct["block_meta"]
    instrs = struct["instrs"]
    bq = struct["bq"]
    EPS = 1e-6
    SUP = 4                    # table-build chunks per super-load

    nc = bacc.Bacc("TRN2", target_bir_lowering=False, debug=False,
                   num_devices=R, num_swdge_queues=NBUK)

    # ---- I/O
    hT16_d = nc.dram_tensor("hT16", [D, n_pad], f16, kind="ExternalInput")
    hT16s_d = nc.dram_tensor("hT16s", [D, npc], f16, kind="ExternalInput")
    W16_d = nc.dram_tensor("W16", [D, D], f16, kind="ExternalInput")
    WT32_d = nc.dram_tensor("WT32", [D, D], f32, kind="ExternalInput")
    A32_d = nc.dram_tensor("A32", [D, 2 * H], f32, kind="ExternalInput")
    iota_d = nc.dram_tensor("iota2d", [P, P], f16, kind="ExternalInput")
    iotac_d = nc.dram_tensor("iota_col", [P, 1], f16, kind="ExternalInput")
    idxg_d = nc.dram_tensor("idxg", [P, nb * P // 16], i16,
                            kind="ExternalInput")
    colv_d = nc.dram_tensor("colv", [P, nb], f16, kind="ExternalInput")
    colf_d = nc.dram_tensor("colfree", [P, nb * P], f16,
                            kind="ExternalInput")
    out_d = nc.dram_tensor("out", [npc, D], f32, kind="ExternalOutput")

    table_d = nc.dram_tensor("table", [n_pad, ROW], f16)

    if debug:
        dbg_tab = nc.dram_tensor("dbg_tab", [n_pad, ROW], f16,
                                 kind="ExternalOutput")
        dbg_x = nc.dram_tensor("dbg_x", [P, MAXBLK * ROW], f16,
                               kind="ExternalOutput")
        dbg_sd = nc.dram_tensor("dbg_sd", [P, MAXBLK * H], f32,
                                kind="ExternalOutput")
        dbg_p = nc.dram_tensor("dbg_p", [P, MAXBLK * H], f32,
                               kind="ExternalOutput")
        dbg_xs = nc.dram_tensor("dbg_xs", [P, MAXBLK * TW], f16,
                                kind="ExternalOutput")
        dbg_dn = nc.dram_tensor("dbg_dn", [P, nt * H], f32,
                                kind="ExternalOutput")
        dbg_sw = nc.dram_tensor("dbg_sw", [P, nt * H], f16,
                                kind="ExternalOutput")

    with tile.TileContext(nc) as tc:
        with (
            tc.tile_pool(name="const", bufs=1) as cpool,
            tc.tile_pool(name="xg", bufs=8) as xpool,
            tc.tile_pool(name="ix", bufs=4) as ixpool,
            tc.tile_pool(name="xs", bufs=4) as xspool,
            tc.tile_pool(name="oh", bufs=4) as ohpool,
            tc.tile_pool(name="sc", bufs=4) as scpool,
            tc.tile_pool(name="fl", bufs=4) as flpool,
            tc.tile_pool(name="agg", bufs=tg + 1, space="PSUM") as aggpsum,
            tc.tile_pool(name="sdp", bufs=2, space="PSUM") as sdpsum,
        ):
            # ================= constants =================
            W16_t = cpool.tile([D, D], f16)
            nc.sync.dma_start(out=W16_t[:], in_=W16_d[:])
            WT32_t = cpool.tile([D, D], f32)
            nc.sync.dma_start(out=WT32_t[:], in_=WT32_d[:])
            A32_t = cpool.tile([D, 2 * H], f32)
            nc.sync.dma_start(out=A32_t[:], in_=A32_d[:])
            iota_t = cpool.tile([P, P], f16)
            nc.sync.dma_start(out=iota_t[:], in_=iota_d[:])
            iotac_t = cpool.tile([P, 1], f16)
            nc.sync.dma_start(out=iotac_t[:], in_=iotac_d[:])
            colv_t = cpool.tile([P, nb], f16)
            nc.sync.dma_start(out=colv_t[:], in_=colv_d[:])
            Wv_t = cpool.tile([D, TW], f16)
            vd_t = cpool.tile([D, H], f16)
            sw_t = cpool.tile([P, nt * H], f16)    # s_dst windows (fp16)

            # ============ prelude (psum shared with edge-phase pools) ======
            with tc.tile_pool(name="pre", bufs=3) as prepool:
                # v = W @ A  (contraction over hidden: lhsT = W^T)
                v_ps = sdpsum.tile([P, MAXBLK * H], f32, space="PSUM",
                                   tag="sdps")
                nc.tensor.matmul(out=v_ps[:D, :2 * H], lhsT=WT32_t[:],
                                 rhs=A32_t[:], start=True, stop=True)
                nc.vector.tensor_copy(out=Wv_t[:, :D], in_=W16_t[:])
                nc.vector.tensor_copy(out=Wv_t[:, D:TW],
                                      in_=v_ps[:D, :H])
                nc.vector.tensor_copy(out=vd_t[:], in_=v_ps[:D, H:2 * H])

                # own s_dst windows: s = h_slice @ v_dst, fp16
                for t in range(nt):
                    hc = prepool.tile([D, P], f16, tag="hc")
                    nc.sync.dma_start(out=hc[:],
                                      in_=hT16s_d[:, t * P:(t + 1) * P])
                    ps = sdpsum.tile([P, MAXBLK * H], f32, space="PSUM",
                                     tag="sdps")
                    nc.tensor.matmul(out=ps[:, :H], lhsT=hc[:], rhs=vd_t[:],
                                     start=True, stop=True)
                    nc.scalar.copy(out=sw_t[:, t * H:(t + 1) * H],
                                   in_=ps[:, :H])

                # replicated table build
                n_ch = n_pad // P
                for s0 in range(0, n_ch, SUP):
                    su = min(SUP, n_ch - s0)
                    hsup = prepool.tile([D, SUP * P], f16, tag="hsup")
                    nc.sync.dma_start(
                        out=hsup[:, :su * P],
                        in_=hT16_d[:, s0 * P:(s0 + su) * P])
                    stg = prepool.tile([P, SUP * ROW], f16, tag="tstage")
                    nc.vector.memset(stg[:], 0.0)
                    stg32 = stg[:].bitcast(f32)       # [P, SUP*128] fp32 view
                    for j in range(su):
                        pt = aggpsum.tile([P, TW], f32, space="PSUM",
                                          tag="agg")
                        nc.tensor.matmul(out=pt[:],
                                         lhsT=hsup[:, j * P:(j + 1) * P],
                                         rhs=Wv_t[:], start=True, stop=True)
                        nc.scalar.copy(out=stg[:, j * ROW:j * ROW + D],
                                       in_=pt[:, :D])
                        nc.vector.tensor_copy(
                            out=stg32[:, j * (ROW // 2) + D // 2:
                                      j * (ROW // 2) + D // 2 + H],
                            in_=pt[:, D:TW])
                    tab_ap = bass.AP(
                        tensor=table_d, offset=s0 * P * ROW,
                        ap=[[ROW, P], [P * ROW, su], [1, ROW]])
                    nc.sync.dma_start(
                        out=tab_ap,
                        in_=stg[:, :su * ROW].rearrange("p (j w) -> p j w",
                                                        w=ROW))

            if debug:
                nc.sync.dma_start(out=dbg_sw[:], in_=sw_t[:])
                nc.sync.dma_start(out=dbg_tab[:], in_=table_d[:])

            # ================= edge phase =================
            buk = n_pad // NBUK
            blocks_seen = np.zeros(nt, np.int64)
            total_blocks = bq.sum(axis=1)          # [nt]
            psum_of_tile = {}

            for k, (qq, b0, nblk) in enumerate(instrs):
                ni = nblk * P
                # ---- idx slice + colfree slice
                ixt = ixpool.tile([P, MAXBLK * 8], i16, tag="ixt")
                nc.sync.dma_start(
                    out=ixt[:, :nblk * 8],
                    in_=idxg_d[:, b0 * 8:(b0 + nblk) * 8])
                cft = ixpool.tile([P, MAXBLK * P], f16, tag="cft")
                nc.sync.dma_start(
                    out=cft[:, :ni],
                    in_=colf_d[:, b0 * P:(b0 + nblk) * P])

                # ---- X gather (512B rows) on queue qq
                xt = xpool.tile([P, MAXBLK, ROW], f16, tag="xt")
                nc.gpsimd.dma_gather(
                    out_ap=xt[:, :nblk, :],
                    in_ap=table_d[qq * buk:(qq + 1) * buk, :],
                    idxs_ap=ixt[:, :nblk * 8],
                    num_idxs=ni, num_idxs_reg=ni, elem_size=ROW,
                    queue_num=qq)

                # ---- one-hot transpose [128w x ni] + s_dst matmuls
                oht = ohpool.tile([P, MAXBLK * P], f16, tag="oht")
                ic_b = iotac_t[:].to_broadcast([P, ni])
                nc.vector.tensor_tensor(out=oht[:, :ni], in0=cft[:, :ni],
                                        in1=ic_b, op=Alu.is_equal)
                sdps = sdpsum.tile([P, MAXBLK * H], f32, space="PSUM",
                                   tag="sdps")
                for j in range(nblk):
                    t = int(block_meta[b0 + j, 0])
                    nc.tensor.matmul(
                        out=sdps[:, j * H:(j + 1) * H],
                        lhsT=oht[:, j * P:(j + 1) * P],
                        rhs=sw_t[:, t * H:(t + 1) * H],
                        start=True, stop=True)

                # ---- scores: ss (fp32 bitcast cols) + sd -> p
                x32 = xt[:].bitcast(f32)       # [P, MAXBLK, ROW//2] fp32
                ss_ap = x32[:, :nblk, D // 2:D // 2 + H]
                ssum = scpool.tile([P, MAXBLK * H], f32, tag="ssum")
                nc.vector.tensor_tensor(
                    out=ssum[:, :nblk * H].rearrange("p (g h) -> p g h", h=H),
                    in0=ss_ap,
                    in1=sdps[:, :nblk * H].rearrange("p (g h) -> p g h", h=H),
                    op=Alu.add)
                ssc = scpool.tile([P, MAXBLK * H], f32, tag="ssc")
                nc.vector.tensor_scalar_mul(
                    out=ssc[:, :nblk * H], in0=ssum[:, :nblk * H],
                    scalar1=NEG)
                nc.vector.tensor_tensor(
                    out=ssum[:, :nblk * H], in0=ssum[:, :nblk * H],
                    in1=ssc[:, :nblk * H], op=Alu.max)
                p32 = scpool.tile([P, MAXBLK * H], f32, tag="p32")
                nc.scalar.activation(out=p32[:, :nblk * H],
                                     in_=ssum[:, :nblk * H], func=Act.Exp)

                if debug and k == 0:
                    nc.sync.dma_start(out=dbg_x[:, :nblk * ROW],
                                      in_=xt[:, :nblk, :].rearrange(
                                          "p g w -> p (g w)"))
                    nc.sync.dma_start(out=dbg_sd[:, :nblk * H],
                                      in_=sdps[:, :nblk * H])
                    nc.sync.dma_start(out=dbg_p[:, :nblk * H],
                                      in_=p32[:, :nblk * H])

                # ---- X' = [wh * p | p]
                xs = xspool.tile([P, MAXBLK * TW], f16, tag="xs")
                xs3 = xs[:].rearrange("p (g w) -> p g w", w=TW)
                nc.scalar.copy(
                    out=xs3[:, :nblk, D:TW],
                    in_=p32[:, :nblk * H])
                wh_in = xt[:, :nblk, :D].rearrange(
                    "p g (h x) -> p g h x", x=HD)
                wh_out = xs3[:, :nblk, :D].rearrange(
                    "p g (h x) -> p g h x", x=HD)
                p_b = xs3[:, :nblk, D:TW].unsqueeze(3).to_broadcast(
                    [P, nblk, H, HD])
                nc.vector.tensor_tensor(out=wh_out, in0=wh_in, in1=p_b,
                                        op=Alu.mult)

                # ---- one-hot (edge-major) for aggregation
                oh = ohpool.tile([P, MAXBLK * P], f16, tag="oh")
                oh3 = oh[:].rearrange("p (g w) -> p g w", w=P)
                col_b = colv_t[:, b0:b0 + nblk].unsqueeze(2).to_broadcast(
                    [P, nblk, P])
                iota_b = iota_t[:].unsqueeze(1).to_broadcast([P, nblk, P])
                nc.vector.tensor_tensor(out=oh3[:, :nblk, :], in0=col_b,
                                        in1=iota_b, op=Alu.is_equal)

                if debug and k == 0:
                    nc.sync.dma_start(out=dbg_xs[:, :nblk * TW],
                                      in_=xs[:, :nblk * TW])

                # ---- aggregation matmuls
                for j in range(nblk):
                    t = int(block_meta[b0 + j, 0])
                    if blocks_seen[t] == 0:
                        psum_of_tile[t] = aggpsum.tile(
                            [P, TW], f32, space="PSUM", tag="agg",
                            name=f"agg_t{t}")
                    pt = psum_of_tile[t]
                    nc.tensor.matmul(
                        out=pt[:],
                        lhsT=oh[:, j * P:(j + 1) * P],
                        rhs=xs[:, j * TW:(j + 1) * TW],
                        start=(blocks_seen[t] == 0),
                        stop=(blocks_seen[t] == total_blocks[t] - 1))
                    blocks_seen[t] += 1
                    if blocks_seen[t] == total_blocks[t]:
                        # ---- flush tile t
                        dn = flpool.tile([P, H], f32, tag="dn")
                        nc.vector.tensor_scalar_max(
                            out=dn[:], in0=pt[:, D:TW], scalar1=EPS)
                        if debug:
                            nc.sync.dma_start(
                                out=dbg_dn[:, t * H:(t + 1) * H], in_=dn[:])
                        rr = flpool.tile([P, H], f32, tag="rr")
                        nc.vector.reciprocal(out=rr[:], in_=dn[:])
                        ot = flpool.tile([P, D], f32, tag="ot")
                        r_b = rr[:].unsqueeze(2).to_broadcast([P, H, HD])
                        u3 = pt[:, :D].rearrange("p (h x) -> p h x", x=HD)
                        o3 = ot[:].rearrange("p (h x) -> p h x", x=HD)
                        nc.vector.tensor_tensor(out=o3, in0=u3, in1=r_b,
                                                op=Alu.mult)
                        nc.sync.dma_start(
                            out=out_d[t * P:(t + 1) * P, :], in_=ot[:])
                        del psum_of_tile[t]

    nc.compile()
    return nc


# ---------------------------------------------------------------- kernel
def kernel(h, edge_index, W, a):
    from concourse.bass_utils import run_bass_kernel_spmd

    npc = _ceil_div(N_NODES, R * P) * P       # 12544
    tg = 4
    per_core, struct = host_prep(h, edge_index, W, a, npc, tg)

    key = ("v2", npc, tg, struct["bq"].tobytes())
    if key not in _PROG_CACHE:
        _PROG_CACHE[key] = build_program(npc, tg, struct)
    nc = _PROG_CACHE[key]

    res = run_bass_kernel_spmd(nc, per_core, list(range(R)))
    out = np.concatenate([res.results[c]["out"] for c in range(R)], axis=0)
    return np.ascontiguousarray(out[:N_NODES]).astype(np.float32)
